# revision 1
# baseline (speedup 1.0000x reference)
"""Trainium2 Bass kernel for GuidedAnchoringRPN loss (nms_detection).

Sharding: core c handles batch b = c//2 and half h = c%2 of every level's
locations.  Each core writes a [128, 12] partial-sum accumulator (per level:
focal-loss sum, shape-loss sum, positive count); the host reduces partials
across cores/partitions and applies the O(1) per-level normalizations.

Device math avoids the reference's [B, nloc, A, G] IoU tensor:
  * IoU is only ever compared (max/argmax/threshold).  With
    asum = area_anchor + area_gt, iou = inter/(asum-inter) is monotone in
    r = inter/asum, so all comparisons run in r-space (iou>=0.5 <=> r>=1/3);
    no per-element union/divide.
  * Guided-anchor pred/target centers coincide, so bounded-IoU dx/dy terms
    vanish; per axis: comp = smoothl1(1 - exp(-|log pw - log tw|)) with
    log tw = log(max(gw_matched,1)), log pw = max(log S + min(sp,4), 0).
  * argmax over GT is recovered via an equality mask against the rowwise
    max, count-normalized to guard exact ties.

All O(B*G) data (GT coords/areas/log-sizes, rasterized loc targets, anchor
tables) is precomputed on host into one [128, 1590] f32 blob per core.
"""

import os
import sys
import numpy as np

sys.path.insert(0, "/opt/trn_rl_repo")

# ---------------------------------------------------------------- constants
STRIDES = (8, 16, 32, 64)
FEAT = ((128, 128), (64, 64), (32, 32), (16, 16))
RATIOS = (0.5, 1.0, 2.0)
OCTAVE_BASE = 8
SCALES_PER_OCT = 3
SQ_SCALE = 8
CENTER_RATIO = 0.2
B, G = 4, 24
NUM_LVLS = 4
V = 9
P = 128

NLOC = tuple(fh * fw for fh, fw in FEAT)
L_ = tuple(n // 2 for n in NLOC)      # per-core locations per level
T_ = tuple(l // P for l in L_)        # (64, 16, 4, 1)
F_ = (8, 8, 4, 1)                     # tiles per instruction group

LVL_OFF = []
_off = 0
for _t in T_:
    LVL_OFF.append(_off)
    _off += 6 * _t                    # CX CY SPW SPH LP CT
CONST_OFF = _off                      # 510
CHW_OFF = [CONST_OFF + l * (2 * V + G * V) for l in range(NUM_LVLS)]
CHH_OFF = [o + V for o in CHW_OFF]
CRAS_OFF = [o + 2 * V for o in CHW_OFF]       # [g, v] layout, v minor
GX1_OFF = CONST_OFF + NUM_LVLS * (2 * V + G * V)   # 1446
GY1_OFF = GX1_OFF + G
GX2_OFF = GY1_OFF + G
GY2_OFF = GX2_OFF + G
LGW_OFF = GY2_OFF + G
LGH_OFF = LGW_OFF + G
NCOLS = LGH_OFF + G                   # 1590

THRESH = 1.0 / 3.0                    # r-space equivalent of iou >= 0.5
LOG_S = [float(np.log(np.float32(SQ_SCALE * s))) for s in STRIDES]

_CACHE = {}
LAST_RESULTS = None


# ---------------------------------------------------------------- host prep
def _f32(x):
    return np.asarray(x, dtype=np.float32)


def _anchor_tables():
    """Per level: half-widths hw[v], half-heights hh[v], area_a[v] (f32)."""
    hw, hh, aa = [], [], []
    for stride in STRIDES:
        bas = []
        for i in range(SCALES_PER_OCT):
            s = stride * OCTAVE_BASE * (2.0 ** (i / SCALES_PER_OCT))
            for r in RATIOS:
                h = s * np.sqrt(r)
                w = s / np.sqrt(r)
                bas.append([-w / 2, -h / 2, w / 2, h / 2])
        ba = np.array(bas, dtype=np.float32)
        hw.append(ba[:, 2].copy())
        hh.append(ba[:, 3].copy())
        aa.append((ba[:, 2] - ba[:, 0]) * (ba[:, 3] - ba[:, 1]))
    return hw, hh, aa


def _host_prep(gt, loc_preds, shape_preds):
    gt = _f32(gt)
    x1, y1, x2, y2 = gt[..., 0], gt[..., 1], gt[..., 2], gt[..., 3]
    bw, bh = x2 - x1, y2 - y1
    cx, cy = (x1 + x2) / 2, (y1 + y2) / 2

    sqrt_area = np.sqrt(np.maximum(bw * bh, np.float32(1e-6)))
    lvl_of = np.clip(
        np.floor(np.log2(np.maximum(sqrt_area, np.float32(1.0)))) - np.float32(2.0),
        0, NUM_LVLS - 1,
    ).astype(np.int32)

    hw_t, hh_t, aa_t = _anchor_tables()
    area_g = (x2 - x1) * (y2 - y1)
    lgw = np.log(np.maximum(x2 - x1, np.float32(1.0)))
    lgh = np.log(np.maximum(y2 - y1, np.float32(1.0)))

    r = CENTER_RATIO
    ct_lvl = []
    for lvl in range(NUM_LVLS):
        (fh, fw), stride = FEAT[lvl], STRIDES[lvl]
        fx1 = np.maximum(0, np.floor((cx - bw * r / 2) / stride)).astype(np.int32)
        fy1 = np.maximum(0, np.floor((cy - bh * r / 2) / stride)).astype(np.int32)
        fx2 = np.minimum(fw, np.floor((cx + bw * r / 2) / stride).astype(np.int32) + 1)
        fy2 = np.minimum(fh, np.floor((cy + bh * r / 2) / stride).astype(np.int32) + 1)
        gxi = np.arange(fw)
        gyi = np.arange(fh)
        mx = (gxi[None, None, :] >= fx1[..., None]) & (gxi[None, None, :] < fx2[..., None])
        my = (gyi[None, None, :] >= fy1[..., None]) & (gyi[None, None, :] < fy2[..., None])
        on = (lvl_of == lvl)[:, :, None, None]
        loc_t = np.any(my[:, :, :, None] & mx[:, :, None, :] & on, axis=1)
        ct_lvl.append(np.float32(1.0) - loc_t.reshape(B, -1).astype(np.float32))

    cx_lvl, cy_lvl = [], []
    for lvl in range(NUM_LVLS):
        (fh, fw), stride = FEAT[lvl], STRIDES[lvl]
        xs = np.arange(fw, dtype=np.float32) * stride + stride / 2
        ys = np.arange(fh, dtype=np.float32) * stride + stride / 2
        cx_lvl.append(np.tile(xs, fh))
        cy_lvl.append(np.repeat(ys, fw))

    def col(a):
        a = _f32(a)
        return np.broadcast_to(a[None, :], (P, a.shape[0]))

    blobs = []
    for core in range(8):
        b, half = core // 2, core % 2
        parts = []
        for lvl in range(NUM_LVLS):
            Tl, Ll = T_[lvl], L_[lvl]
            sel = slice(half * Ll, (half + 1) * Ll)

            def tilecols(flat):
                return _f32(flat)[sel].reshape(Tl, P).T

            sp = shape_preds[lvl][b]
            parts += [
                tilecols(cx_lvl[lvl]),
                tilecols(cy_lvl[lvl]),
                tilecols(_f32(sp[0]).reshape(-1)),
                tilecols(_f32(sp[1]).reshape(-1)),
                tilecols(_f32(loc_preds[lvl][b, 0]).reshape(-1)),
                tilecols(ct_lvl[lvl][b]),
            ]
        for lvl in range(NUM_LVLS):
            ras = np.float32(1.0) / (aa_t[lvl][None, :] + area_g[b][:, None])  # [G,V]
            parts += [col(hw_t[lvl]), col(hh_t[lvl]), col(ras.reshape(-1))]
        parts += [col(gt[b, :, 0]), col(gt[b, :, 1]), col(gt[b, :, 2]), col(gt[b, :, 3])]
        parts += [col(lgw[b]), col(lgh[b])]
        blob = np.ascontiguousarray(np.concatenate(parts, axis=1), dtype=np.float32)
        assert blob.shape == (P, NCOLS), blob.shape
        blobs.append(blob)
    return blobs


# ---------------------------------------------------------------- device
def _build():
    if "nc" in _CACHE:
        return _CACHE["nc"]
    import concourse.bass as bass  # noqa: F401
    from concourse import bacc, mybir, tile

    f32 = mybir.dt.float32
    AL = mybir.AluOpType
    AF = mybir.ActivationFunctionType
    AX = mybir.AxisListType

    nc = bacc.Bacc("TRN2", target_bir_lowering=False, debug=False, num_devices=8)
    X = nc.declare_dram_parameter("x", [P, NCOLS], f32, isOutput=False)
    OUT = nc.declare_dram_parameter("out", [P, 12], f32, isOutput=True)

    with tile.TileContext(nc) as tc:
        with tc.tile_pool(name="io", bufs=1) as iop, \
             tc.tile_pool(name="big", bufs=2) as bigp, \
             tc.tile_pool(name="sm", bufs=2) as smp, \
             tc.tile_pool(name="pb", bufs=2) as pbp, \
             tc.tile_pool(name="keep", bufs=1) as kp:

            XS = iop.tile([P, NCOLS], f32, name="XS", tag="XS")
            nc.sync.dma_start(out=XS[:], in_=X[:])
            ACC = iop.tile([P, 12], f32, name="ACC", tag="ACC")

            gx1 = XS[:, GX1_OFF:GX1_OFF + G]
            gy1 = XS[:, GY1_OFF:GY1_OFF + G]
            gx2 = XS[:, GX2_OFF:GX2_OFF + G]
            gy2 = XS[:, GY2_OFF:GY2_OFF + G]
            lgw = XS[:, LGW_OFF:LGW_OFF + G]
            lgh = XS[:, LGH_OFF:LGH_OFF + G]

            def bcg(ap, F):      # [128,G] -> [128,F,G]
                return ap.unsqueeze(1).broadcast_to((P, F, G))

            def bcc(ap, F):      # [128,F] -> [128,F,G]
                return ap.unsqueeze(2).broadcast_to((P, F, G))

            def bcv(ap, F):      # [128,V] -> [128,F,G,V]
                return ap.unsqueeze(1).unsqueeze(1).broadcast_to((P, F, G, V))

            def bcd(ap, F):      # [128,F,G] -> [128,F,G,V]
                return ap.unsqueeze(3).broadcast_to((P, F, G, V))

            def bcr(ap, F):      # [128,G,V] -> [128,F,G,V]
                return ap.unsqueeze(1).broadcast_to((P, F, G, V))

            for lvl in range(NUM_LVLS):
                T, F = T_[lvl], F_[lvl]
                base = LVL_OFF[lvl]
                cxA = XS[:, base + 0 * T: base + 1 * T]
                cyA = XS[:, base + 1 * T: base + 2 * T]
                spwA = XS[:, base + 2 * T: base + 3 * T]
                sphA = XS[:, base + 3 * T: base + 4 * T]
                lpA = XS[:, base + 4 * T: base + 5 * T]
                ctA = XS[:, base + 5 * T: base + 6 * T]
                hw9 = XS[:, CHW_OFF[lvl]:CHW_OFF[lvl] + V]
                hh9 = XS[:, CHH_OFF[lvl]:CHH_OFF[lvl] + V]
                ras = XS[:, CRAS_OFF[lvl]:CRAS_OFF[lvl] + G * V].rearrange(
                    "p (g v) -> p g v", v=V)

                MLW = kp.tile([P, T], f32, name=f"mlw{lvl}", tag=f"mlw{lvl}")
                MLH = kp.tile([P, T], f32, name=f"mlh{lvl}", tag=f"mlh{lvl}")
                POS = kp.tile([P, T], f32, name=f"pos{lvl}", tag=f"pos{lvl}")

                for f0 in range(0, T, F):
                    cx = cxA[:, f0:f0 + F]
                    cy = cyA[:, f0:f0 + F]

                    dx1 = smp.tile([P, F, G], f32, name="dx1", tag="dx1")
                    dx2 = smp.tile([P, F, G], f32, name="dx2", tag="dx2")
                    dy1 = smp.tile([P, F, G], f32, name="dy1", tag="dy1")
                    dy2 = smp.tile([P, F, G], f32, name="dy2", tag="dy2")
                    nc.gpsimd.tensor_tensor(out=dx1[:, :F], in0=bcc(cx, F), in1=bcg(gx1, F), op=AL.subtract)
                    nc.gpsimd.tensor_tensor(out=dx2[:, :F], in0=bcg(gx2, F), in1=bcc(cx, F), op=AL.subtract)
                    nc.gpsimd.tensor_tensor(out=dy1[:, :F], in0=bcc(cy, F), in1=bcg(gy1, F), op=AL.subtract)
                    nc.gpsimd.tensor_tensor(out=dy2[:, :F], in0=bcg(gy2, F), in1=bcc(cy, F), op=AL.subtract)

                    t1 = bigp.tile([P, F, G, V], f32, name="t1", tag="t1")
                    t2 = bigp.tile([P, F, G, V], f32, name="t2", tag="t2")
                    ix = bigp.tile([P, F, G, V], f32, name="ix", tag="ix")
                    t3 = bigp.tile([P, F, G, V], f32, name="t3", tag="t3")
                    t4 = bigp.tile([P, F, G, V], f32, name="t4", tag="t4")
                    iy = bigp.tile([P, F, G, V], f32, name="iy", tag="iy")
                    iy2 = bigp.tile([P, F, G, V], f32, name="iy2", tag="iy2")
                    rr = bigp.tile([P, F, G, V], f32, name="rr", tag="rr")

                    nc.vector.tensor_tensor(out=t3[:, :F], in0=bcv(hh9, F), in1=bcd(dy1[:, :F], F), op=AL.min)
                    nc.vector.tensor_tensor(out=t4[:, :F], in0=bcv(hh9, F), in1=bcd(dy2[:, :F], F), op=AL.min)
                    nc.gpsimd.tensor_tensor(out=iy[:, :F], in0=t3[:, :F], in1=t4[:, :F], op=AL.add)
                    nc.vector.tensor_tensor(out=t1[:, :F], in0=bcv(hw9, F), in1=bcd(dx1[:, :F], F), op=AL.min)
                    nc.vector.tensor_tensor(out=t2[:, :F], in0=bcv(hw9, F), in1=bcd(dx2[:, :F], F), op=AL.min)
                    nc.gpsimd.tensor_tensor(out=ix[:, :F], in0=t1[:, :F], in1=t2[:, :F], op=AL.add)
                    nc.gpsimd.tensor_tensor(out=iy2[:, :F], in0=iy[:, :F], in1=bcr(ras, F), op=AL.mult)
                    # rr = max(ix, 0) * (iy * ras); negative iy never crosses
                    # the threshold nor beats any positive candidate.
                    nc.vector.scalar_tensor_tensor(
                        out=rr[:, :F], in0=ix[:, :F], scalar=0.0, in1=iy2[:, :F],
                        op0=AL.max, op1=AL.mult)

                    miou = smp.tile([P, F, G], f32, name="miou", tag="miou")
                    nc.vector.reduce_max(out=miou[:, :F], in_=rr[:, :F], axis=AX.X)
                    maxg = smp.tile([P, F], f32, name="maxg", tag="maxg")
                    nc.vector.reduce_max(out=maxg[:, :F], in_=miou[:, :F], axis=AX.X)
                    nc.gpsimd.tensor_single_scalar(
                        out=POS[:, f0:f0 + F], in_=maxg[:, :F], scalar=THRESH, op=AL.is_ge)

                    eq = smp.tile([P, F, G], f32, name="eq", tag="eq")
                    nc.vector.tensor_tensor(
                        out=eq[:, :F], in0=miou[:, :F],
                        in1=maxg[:, :F].unsqueeze(2).broadcast_to((P, F, G)), op=AL.is_equal)
                    cnt = smp.tile([P, F], f32, name="cnt", tag="cnt")
                    nc.vector.reduce_sum(out=cnt[:, :F], in_=eq[:, :F], axis=AX.X)
                    wn = smp.tile([P, F, G], f32, name="wn", tag="wn")
                    hn = smp.tile([P, F, G], f32, name="hn", tag="hn")
                    nc.gpsimd.tensor_tensor(out=wn[:, :F], in0=eq[:, :F], in1=bcg(lgw, F), op=AL.mult)
                    nc.gpsimd.tensor_tensor(out=hn[:, :F], in0=eq[:, :F], in1=bcg(lgh, F), op=AL.mult)
                    wnum = smp.tile([P, F], f32, name="wnum", tag="wnum")
                    hnum = smp.tile([P, F], f32, name="hnum", tag="hnum")
                    nc.vector.reduce_sum(out=wnum[:, :F], in_=wn[:, :F], axis=AX.X)
                    nc.vector.reduce_sum(out=hnum[:, :F], in_=hn[:, :F], axis=AX.X)
                    rc = smp.tile([P, F], f32, name="rc", tag="rc")
                    nc.vector.reciprocal(out=rc[:, :F], in_=cnt[:, :F])
                    nc.gpsimd.tensor_tensor(out=MLW[:, f0:f0 + F], in0=wnum[:, :F], in1=rc[:, :F], op=AL.mult)
                    nc.gpsimd.tensor_tensor(out=MLH[:, f0:f0 + F], in0=hnum[:, :F], in1=rc[:, :F], op=AL.mult)

                # ---------------- phase B: focal + shape loss tails ----------
                sg = pbp.tile([P, T], f32, name="sg", tag="sg")
                nc.scalar.activation(out=sg[:], in_=lpA, func=AF.Sigmoid)
                a1 = pbp.tile([P, T], f32, name="a1", tag="a1")
                nc.scalar.activation(out=a1[:], in_=sg[:], func=AF.Copy, bias=1.0, scale=-2.0)
                ptm = pbp.tile([P, T], f32, name="ptm", tag="ptm")
                nc.gpsimd.tensor_tensor(out=ptm[:], in0=ctA, in1=a1[:], op=AL.mult)
                pt = pbp.tile([P, T], f32, name="pt", tag="pt")
                nc.gpsimd.tensor_tensor(out=pt[:], in0=ptm[:], in1=sg[:], op=AL.add)
                ptc = pbp.tile([P, T], f32, name="ptc", tag="ptc")
                nc.gpsimd.tensor_single_scalar(out=ptc[:], in_=pt[:], scalar=1e-6, op=AL.max)
                lg = pbp.tile([P, T], f32, name="lg", tag="lg")
                nc.scalar.activation(out=lg[:], in_=ptc[:], func=AF.Ln)
                om2 = pbp.tile([P, T], f32, name="om2", tag="om2")
                nc.scalar.activation(out=om2[:], in_=pt[:], func=AF.Square, bias=1.0, scale=-1.0)
                s1 = pbp.tile([P, T], f32, name="s1", tag="s1")
                nc.gpsimd.tensor_tensor(out=s1[:], in0=om2[:], in1=lg[:], op=AL.mult)
                at = pbp.tile([P, T], f32, name="at", tag="at")
                nc.gpsimd.tensor_scalar(at[:], ctA, 0.5, 0.25, AL.mult, AL.add)
                s2 = pbp.tile([P, T], f32, name="s2", tag="s2")
                nc.gpsimd.tensor_tensor(out=s2[:], in0=at[:], in1=s1[:], op=AL.mult)
                nc.vector.reduce_sum(
                    out=ACC[:, 3 * lvl:3 * lvl + 1], in_=s2[:], axis=AX.X)

                slo = []
                for ax, (spA, ML) in enumerate(((spwA, MLW), (sphA, MLH))):
                    lpw = pbp.tile([P, T], f32, name=f"lpw{ax}", tag=f"lpw{ax}")
                    nc.gpsimd.tensor_scalar(lpw[:], spA, 4.0, LOG_S[lvl], AL.min, AL.add)
                    dwm = pbp.tile([P, T], f32, name=f"dwm{ax}", tag=f"dwm{ax}")
                    nc.vector.scalar_tensor_tensor(
                        out=dwm[:], in0=lpw[:], scalar=0.0, in1=ML[:],
                        op0=AL.max, op1=AL.subtract)
                    dw = pbp.tile([P, T], f32, name=f"dw{ax}", tag=f"dw{ax}")
                    nc.scalar.activation(out=dw[:], in_=dwm[:], func=AF.Abs)
                    ee = pbp.tile([P, T], f32, name=f"ee{ax}", tag=f"ee{ax}")
                    nc.scalar.activation(out=ee[:], in_=dw[:], func=AF.Exp, scale=-1.0)
                    c1 = pbp.tile([P, T], f32, name=f"c1{ax}", tag=f"c1{ax}")
                    nc.gpsimd.tensor_single_scalar(out=c1[:], in_=ee[:], scalar=0.8, op=AL.max)
                    u2s = pbp.tile([P, T], f32, name=f"u2s{ax}", tag=f"u2s{ax}")
                    nc.scalar.activation(out=u2s[:], in_=c1[:], func=AF.Square, bias=1.0, scale=-1.0)
                    d1 = pbp.tile([P, T], f32, name=f"d1{ax}", tag=f"d1{ax}")
                    nc.gpsimd.tensor_tensor(out=d1[:], in0=c1[:], in1=ee[:], op=AL.subtract)
                    sl = pbp.tile([P, T], f32, name=f"sl{ax}", tag=f"sl{ax}")
                    nc.vector.scalar_tensor_tensor(
                        out=sl[:], in0=u2s[:], scalar=2.5, in1=d1[:],
                        op0=AL.mult, op1=AL.add)
                    slo.append(sl)
                ssum = pbp.tile([P, T], f32, name="ssum", tag="ssum")
                nc.gpsimd.tensor_tensor(out=ssum[:], in0=slo[0][:], in1=slo[1][:], op=AL.add)
                spm = pbp.tile([P, T], f32, name="spm", tag="spm")
                nc.gpsimd.tensor_tensor(out=spm[:], in0=ssum[:], in1=POS[:], op=AL.mult)
                nc.vector.reduce_sum(
                    out=ACC[:, 3 * lvl + 1:3 * lvl + 2], in_=spm[:], axis=AX.X)
                nc.vector.reduce_sum(out=ACC[:, 3 * lvl + 2:3 * lvl + 3], in_=POS[:], axis=AX.X)

            nc.sync.dma_start(out=OUT[:], in_=ACC[:])
    nc.compile()
    _CACHE["nc"] = nc
    return nc


# ---------------------------------------------------------------- emulation
def _emulate_core(blob):
    """numpy mirror of the device program, one core blob -> [128,12]."""
    X = blob.astype(np.float32)
    acc = np.zeros((P, 12), np.float32)
    gx1 = X[:, GX1_OFF:GX1_OFF + G]
    gy1 = X[:, GY1_OFF:GY1_OFF + G]
    gx2 = X[:, GX2_OFF:GX2_OFF + G]
    gy2 = X[:, GY2_OFF:GY2_OFF + G]
    lgw = X[:, LGW_OFF:LGW_OFF + G]
    lgh = X[:, LGH_OFF:LGH_OFF + G]
    for lvl in range(NUM_LVLS):
        T = T_[lvl]
        base = LVL_OFF[lvl]
        cx = X[:, base:base + T]
        cy = X[:, base + T:base + 2 * T]
        spw = X[:, base + 2 * T:base + 3 * T]
        sph = X[:, base + 3 * T:base + 4 * T]
        lp = X[:, base + 4 * T:base + 5 * T]
        ct = X[:, base + 5 * T:base + 6 * T]
        hw9 = X[:, CHW_OFF[lvl]:CHW_OFF[lvl] + V]
        hh9 = X[:, CHH_OFF[lvl]:CHH_OFF[lvl] + V]
        ras = X[:, CRAS_OFF[lvl]:CRAS_OFF[lvl] + G * V].reshape(P, G, V)

        dx1 = cx[:, :, None] - gx1[:, None, :]
        dx2 = gx2[:, None, :] - cx[:, :, None]
        dy1 = cy[:, :, None] - gy1[:, None, :]
        dy2 = gy2[:, None, :] - cy[:, :, None]
        t1 = np.minimum(hw9[:, None, None, :], dx1[..., None])
        t2 = np.minimum(hw9[:, None, None, :], dx2[..., None])
        ixv = t1 + t2
        t3 = np.minimum(hh9[:, None, None, :], dy1[..., None])
        t4 = np.minimum(hh9[:, None, None, :], dy2[..., None])
        iyv = t3 + t4
        iy2 = iyv * ras[:, None, :, :]
        rrv = np.maximum(ixv, np.float32(0)) * iy2
        miou = rrv.max(axis=3)
        maxg = miou.max(axis=2)
        pos = (maxg >= np.float32(THRESH)).astype(np.float32)
        eq = (miou == maxg[:, :, None]).astype(np.float32)
        cnt = eq.sum(axis=2, dtype=np.float32)
        wnum = (eq * lgw[:, None, :]).sum(axis=2, dtype=np.float32)
        hnum = (eq * lgh[:, None, :]).sum(axis=2, dtype=np.float32)
        rcv = np.float32(1.0) / cnt
        mlw = wnum * rcv
        mlh = hnum * rcv

        # phase B
        sg = np.float32(1.0) / (np.float32(1.0) + np.exp(-lp, dtype=np.float32))
        a1 = np.float32(1.0) - np.float32(2.0) * sg
        pt = ct * a1 + sg
        ptc = np.maximum(pt, np.float32(1e-6))
        lgv = np.log(ptc, dtype=np.float32)
        om2 = np.square(np.float32(1.0) - pt)
        s1 = om2 * lgv
        at = np.float32(0.25) + np.float32(0.5) * ct
        acc[:, 3 * lvl] = (at * s1).sum(axis=1, dtype=np.float32)

        sls = []
        for spA, ML in ((spw, mlw), (sph, mlh)):
            lpw = np.minimum(spA, np.float32(4.0)) + np.float32(LOG_S[lvl])
            dwm = np.maximum(lpw, np.float32(0.0)) - ML
            dwv = np.abs(dwm)
            ee = np.exp(-dwv, dtype=np.float32)
            c1 = np.maximum(ee, np.float32(0.8))
            u2s = np.square(np.float32(1.0) - c1)
            d1 = c1 - ee
            sls.append(np.float32(2.5) * u2s + d1)
        ssum = sls[0] + sls[1]
        acc[:, 3 * lvl + 1] = (ssum * pos).sum(axis=1, dtype=np.float32)
        acc[:, 3 * lvl + 2] = pos.sum(axis=1, dtype=np.float32)
    return acc


# ---------------------------------------------------------------- entry
def _combine(parts):
    s = parts.astype(np.float64).sum(axis=(0, 1))  # [12]
    loc, shp = 0.0, 0.0
    for lvl in range(NUM_LVLS):
        fh, fw = FEAT[lvl]
        loc += (-s[3 * lvl]) / (B * fh * fw)
        shp += s[3 * lvl + 1] / max(4.0 * s[3 * lvl + 2], 1.0)
    return np.array((loc + shp) / NUM_LVLS, dtype=np.float32)


def kernel(**inputs):
    global LAST_RESULTS
    gt = np.asarray(inputs["gt_boxes"], dtype=np.float32)
    loc_preds = [np.asarray(inputs[f"loc_pred{l}"], dtype=np.float32) for l in range(NUM_LVLS)]
    shape_preds = [np.asarray(inputs[f"shape_pred{l}"], dtype=np.float32) for l in range(NUM_LVLS)]
    blobs = _host_prep(gt, loc_preds, shape_preds)

    if os.environ.get("KERNEL_EMULATE"):
        parts = np.stack([_emulate_core(b) for b in blobs])
        return _combine(parts)

    nc = _build()
    from concourse.bass_utils import run_bass_kernel_spmd
    res = run_bass_kernel_spmd(
        nc, [{"x": b} for b in blobs], core_ids=list(range(8)),
        trace=bool(os.environ.get("BASS_TRACE")))
    LAST_RESULTS = res
    parts = np.stack([r["out"] for r in res.results])
    return _combine(parts)



# revision 2
# speedup vs baseline: 6.2267x; 6.2267x over previous
"""Trainium2 Bass kernel for GuidedAnchoringRPN loss (nms_detection).

Sharding: core c handles batch b = c//2 and half h = c%2 of every level's
locations.  Each core writes a [128, 12] partial-sum accumulator (per level:
focal-loss sum, shape-loss sum, positive count); the host reduces partials
across cores/partitions and applies the O(1) per-level normalizations.

Device math avoids the reference's [B, nloc, A, G] IoU tensor:
  * IoU is only ever compared (max/argmax/threshold).  With
    asum = area_anchor + area_gt, iou = inter/(asum-inter) is monotone in
    r = inter/asum, so all comparisons run in r-space (iou>=0.5 <=> r>=1/3);
    no per-element union/divide.
  * Guided-anchor pred/target centers coincide, so bounded-IoU dx/dy terms
    vanish; per axis: comp = smoothl1(1 - exp(-|log pw - log tw|)) with
    log tw = log(max(gw_matched,1)), log pw = max(log S + min(sp,4), 0).
  * argmax over GT is recovered via an equality mask against the rowwise
    max, count-normalized to guard exact ties.

Wall-clock (the graded metric) is dominated by dispatch overhead, not
device cycles, so the entry point is built around a cached jitted
shard_map dispatcher:
  * the jax.jit(shard_map(_bass_exec)) callable is built once per process
    (run_bass_kernel_spmd re-traces and re-lowers it on every call);
  * static per-location tables (anchor centers, anchor half-sizes) are
    SPMD-uniform -- the only cross-core difference is a +512*(core%2)
    shift on cy, shipped via xc -- and live in a device-resident sharded
    array that is device_put exactly once;
  * per-call payload is just the predictions + rasterized loc-targets in
    bf16 ([1024, 340], ~0.7 MB) and a tiny per-core scalar row xc
    ([8, 1024] f32) that the device broadcasts across partitions with
    log-doubling SBUF DMAs;
  * outputs are written fully by the kernel, so no donated zero buffers.
"""

import os
import sys
import numpy as np

sys.path.insert(0, "/opt/trn_rl_repo")

# ---------------------------------------------------------------- constants
STRIDES = (8, 16, 32, 64)
FEAT = ((128, 128), (64, 64), (32, 32), (16, 16))
RATIOS = (0.5, 1.0, 2.0)
OCTAVE_BASE = 8
SCALES_PER_OCT = 3
SQ_SCALE = 8
CENTER_RATIO = 0.2
B, G = 4, 24
NUM_LVLS = 4
V = 9
P = 128
N_CORES = 8

NLOC = tuple(fh * fw for fh, fw in FEAT)
L_ = tuple(n // 2 for n in NLOC)      # per-core locations per level
T_ = tuple(l // P for l in L_)        # (64, 16, 4, 1)
F_ = (8, 8, 4, 1)                     # tiles per instruction group
SUM_T = sum(T_)                       # 85

# static xs layout: per level CX(T), CYU(T); then per level HW9(9), HH9(9)
SX_OFF = []
_o = 0
for _t in T_:
    SX_OFF.append(_o)
    _o += 2 * _t
SHW_OFF = [2 * SUM_T + 18 * l for l in range(NUM_LVLS)]
SHH_OFF = [o + V for o in SHW_OFF]
NSC = 2 * SUM_T + 18 * NUM_LVLS       # 242

# dynamic xp layout (bf16): per level SPW(T), SPH(T), LP(T), CT(T)
PX_OFF = []
_o = 0
for _t in T_:
    PX_OFF.append(_o)
    _o += 4 * _t
NPC = 4 * SUM_T                       # 340

# per-core scalar row xc (f32): RAS per level (216 each), then GX1 GY1 GX2
# GY2 LGW LGH (24 each), then CYOFF (1), padded to 1024
CRAS_OFF = [216 * l for l in range(NUM_LVLS)]
GX1_OFF = 864
GY1_OFF = GX1_OFF + G
GX2_OFF = GY1_OFF + G
GY2_OFF = GX2_OFF + G
LGW_OFF = GY2_OFF + G
LGH_OFF = LGW_OFF + G
CYOFF_COL = LGH_OFF + G               # 1008
NCC = 1024

THRESH = 1.0 / 3.0                    # r-space equivalent of iou >= 0.5
LOG_S = [float(np.log(np.float32(SQ_SCALE * s))) for s in STRIDES]

_CACHE = {}
LAST_RESULTS = None


# ---------------------------------------------------------------- host prep
def _f32(x):
    return np.asarray(x, dtype=np.float32)


def _anchor_tables():
    """Per level: half-widths hw[v], half-heights hh[v], area_a[v] (f32)."""
    hw, hh, aa = [], [], []
    for stride in STRIDES:
        bas = []
        for i in range(SCALES_PER_OCT):
            s = stride * OCTAVE_BASE * (2.0 ** (i / SCALES_PER_OCT))
            for r in RATIOS:
                h = s * np.sqrt(r)
                w = s / np.sqrt(r)
                bas.append([-w / 2, -h / 2, w / 2, h / 2])
        ba = np.array(bas, dtype=np.float32)
        hw.append(ba[:, 2].copy())
        hh.append(ba[:, 3].copy())
        aa.append((ba[:, 2] - ba[:, 0]) * (ba[:, 3] - ba[:, 1]))
    return hw, hh, aa


def _static_block():
    """[128, NSC] static table, identical on every core (half-0 cy)."""
    if "xs_blk" in _CACHE:
        return _CACHE["xs_blk"]
    hw_t, hh_t, _ = _anchor_tables()
    blk = np.zeros((P, NSC), np.float32)
    for lvl in range(NUM_LVLS):
        (fh, fw), stride = FEAT[lvl], STRIDES[lvl]
        Tl, Ll = T_[lvl], L_[lvl]
        xs = np.arange(fw, dtype=np.float32) * stride + stride / 2
        ys = np.arange(fh, dtype=np.float32) * stride + stride / 2
        cx_full = np.tile(xs, fh)
        cy_full = np.repeat(ys, fw)
        cx0 = cx_full[:Ll].reshape(Tl, P).T
        cy0 = cy_full[:Ll].reshape(Tl, P).T
        # the half-1 slice differs from half-0 by exactly +512 on cy and
        # matches on cx at every level (fh/2 * stride == 512)
        blk[:, SX_OFF[lvl]:SX_OFF[lvl] + Tl] = cx0
        blk[:, SX_OFF[lvl] + Tl:SX_OFF[lvl] + 2 * Tl] = cy0
        blk[:, SHW_OFF[lvl]:SHW_OFF[lvl] + V] = hw_t[lvl][None, :]
        blk[:, SHH_OFF[lvl]:SHH_OFF[lvl] + V] = hh_t[lvl][None, :]
    _CACHE["xs_blk"] = blk
    return blk


def _rasterize_ct(gt, lvl_of):
    """ct = 1 - loc_target per (b, lvl); [B][lvl] -> [fh*fw] f32."""
    x1, y1, x2, y2 = gt[..., 0], gt[..., 1], gt[..., 2], gt[..., 3]
    bw, bh = x2 - x1, y2 - y1
    cx, cy = (x1 + x2) / 2, (y1 + y2) / 2
    r = np.float32(CENTER_RATIO)
    ct = [[None] * NUM_LVLS for _ in range(B)]
    for lvl in range(NUM_LVLS):
        (fh, fw), stride = FEAT[lvl], STRIDES[lvl]
        s = np.float32(stride)
        fx1 = np.maximum(0, np.floor((cx - bw * r / 2) / s)).astype(np.int64)
        fy1 = np.maximum(0, np.floor((cy - bh * r / 2) / s)).astype(np.int64)
        fx2 = np.minimum(fw, np.floor((cx + bw * r / 2) / s).astype(np.int64) + 1)
        fy2 = np.minimum(fh, np.floor((cy + bh * r / 2) / s).astype(np.int64) + 1)
        on = lvl_of == lvl
        for b in range(B):
            m = np.zeros((fh, fw), np.float32)
            for g in np.nonzero(on[b])[0]:
                m[fy1[b, g]:fy2[b, g], fx1[b, g]:fx2[b, g]] = 1.0
            ct[b][lvl] = np.float32(1.0) - m.reshape(-1)
    return ct


def _host_prep(gt, loc_preds, shape_preds):
    import ml_dtypes

    gt = _f32(gt)
    x1, y1, x2, y2 = gt[..., 0], gt[..., 1], gt[..., 2], gt[..., 3]
    bw, bh = x2 - x1, y2 - y1

    sqrt_area = np.sqrt(np.maximum(bw * bh, np.float32(1e-6)))
    lvl_of = np.clip(
        np.floor(np.log2(np.maximum(sqrt_area, np.float32(1.0)))) - np.float32(2.0),
        0, NUM_LVLS - 1,
    ).astype(np.int32)

    _, _, aa_t = _anchor_tables()
    area_g = bw * bh
    lgw = np.log(np.maximum(bw, np.float32(1.0)))
    lgh = np.log(np.maximum(bh, np.float32(1.0)))
    ct = _rasterize_ct(gt, lvl_of)

    # xp: [B, 2, P, NPC] -> [N_CORES*P, NPC] bf16
    xp = np.empty((B, 2, P, NPC), np.float32)
    for lvl in range(NUM_LVLS):
        Tl, o = T_[lvl], PX_OFF[lvl]
        sp = shape_preds[lvl].reshape(B, 2, 2, Tl, P)       # [B, ch, half, T, p]
        xp[:, :, :, o:o + Tl] = sp[:, 0].transpose(0, 1, 3, 2)
        xp[:, :, :, o + Tl:o + 2 * Tl] = sp[:, 1].transpose(0, 1, 3, 2)
        lp = loc_preds[lvl].reshape(B, 2, Tl, P)
        xp[:, :, :, o + 2 * Tl:o + 3 * Tl] = lp.transpose(0, 1, 3, 2)
        for b in range(B):
            c = ct[b][lvl].reshape(2, Tl, P)
            xp[b, :, :, o + 3 * Tl:o + 4 * Tl] = c.transpose(0, 2, 1)
    xp_bf = np.ascontiguousarray(xp.reshape(N_CORES * P, NPC)).astype(
        ml_dtypes.bfloat16)

    # xc: [N_CORES, NCC] f32
    xc = np.zeros((N_CORES, NCC), np.float32)
    for b in range(B):
        row = np.zeros(NCC, np.float32)
        for lvl in range(NUM_LVLS):
            ras = np.float32(1.0) / (aa_t[lvl][None, :] + area_g[b][:, None])
            row[CRAS_OFF[lvl]:CRAS_OFF[lvl] + G * V] = ras.reshape(-1)
        row[GX1_OFF:GX1_OFF + G] = gt[b, :, 0]
        row[GY1_OFF:GY1_OFF + G] = gt[b, :, 1]
        row[GX2_OFF:GX2_OFF + G] = gt[b, :, 2]
        row[GY2_OFF:GY2_OFF + G] = gt[b, :, 3]
        row[LGW_OFF:LGW_OFF + G] = lgw[b]
        row[LGH_OFF:LGH_OFF + G] = lgh[b]
        xc[2 * b] = row
        xc[2 * b + 1] = row
        xc[2 * b + 1, CYOFF_COL] = 512.0
    return xp_bf, xc


# ---------------------------------------------------------------- device
def _build():
    if "nc" in _CACHE:
        return _CACHE["nc"]
    import concourse.bass as bass  # noqa: F401
    from concourse import bacc, mybir, tile

    f32 = mybir.dt.float32
    bf16 = mybir.dt.bfloat16
    AL = mybir.AluOpType
    AF = mybir.ActivationFunctionType
    AX = mybir.AxisListType

    nc = bacc.Bacc("TRN2", target_bir_lowering=False, debug=False,
                   num_devices=N_CORES)
    XSP = nc.declare_dram_parameter("xs", [P, NSC], f32, isOutput=False)
    XPP = nc.declare_dram_parameter("xp", [P, NPC], bf16, isOutput=False)
    XCP = nc.declare_dram_parameter("xc", [1, NCC], f32, isOutput=False)
    OUT = nc.declare_dram_parameter("out", [P, 12], f32, isOutput=True)

    with tile.TileContext(nc) as tc:
        with tc.tile_pool(name="io", bufs=1) as iop, \
             tc.tile_pool(name="big", bufs=2) as bigp, \
             tc.tile_pool(name="sm", bufs=2) as smp, \
             tc.tile_pool(name="pb", bufs=2) as pbp, \
             tc.tile_pool(name="keep", bufs=1) as kp:

            XS = iop.tile([P, NSC], f32, name="XS", tag="XS")
            nc.sync.dma_start(out=XS[:], in_=XSP[:])
            XPB = iop.tile([P, NPC], bf16, name="XPB", tag="XPB")
            nc.sync.dma_start(out=XPB[:], in_=XPP[:])
            XCB = iop.tile([P, NCC], f32, name="XCB", tag="XCB")
            nc.sync.dma_start(out=XCB[0:1, :], in_=XCP[:])
            # broadcast xc across partitions by log-doubling
            k = 1
            while k < P:
                nc.sync.dma_start(out=XCB[k:2 * k, :], in_=XCB[0:k, :])
                k *= 2

            XPF = iop.tile([P, NPC], f32, name="XPF", tag="XPF")
            nc.scalar.activation(out=XPF[:], in_=XPB[:], func=AF.Copy)

            # cy adjusted by the per-core +512*(core%2) offset
            CYA = iop.tile([P, SUM_T], f32, name="CYA", tag="CYA")
            cyo = XCB[:, CYOFF_COL:CYOFF_COL + 1]
            cy_pos = []
            _o = 0
            for lvl in range(NUM_LVLS):
                Tl = T_[lvl]
                cy_pos.append(_o)
                nc.gpsimd.tensor_tensor(
                    out=CYA[:, _o:_o + Tl],
                    in0=XS[:, SX_OFF[lvl] + Tl:SX_OFF[lvl] + 2 * Tl],
                    in1=cyo.broadcast_to((P, Tl)), op=AL.add)
                _o += Tl

            ACC = iop.tile([P, 12], f32, name="ACC", tag="ACC")

            gx1 = XCB[:, GX1_OFF:GX1_OFF + G]
            gy1 = XCB[:, GY1_OFF:GY1_OFF + G]
            gx2 = XCB[:, GX2_OFF:GX2_OFF + G]
            gy2 = XCB[:, GY2_OFF:GY2_OFF + G]
            lgw = XCB[:, LGW_OFF:LGW_OFF + G]
            lgh = XCB[:, LGH_OFF:LGH_OFF + G]

            def bcg(ap, F):      # [128,G] -> [128,F,G]
                return ap.unsqueeze(1).broadcast_to((P, F, G))

            def bcc(ap, F):      # [128,F] -> [128,F,G]
                return ap.unsqueeze(2).broadcast_to((P, F, G))

            def bcv(ap, F):      # [128,V] -> [128,F,G,V]
                return ap.unsqueeze(1).unsqueeze(1).broadcast_to((P, F, G, V))

            def bcd(ap, F):      # [128,F,G] -> [128,F,G,V]
                return ap.unsqueeze(3).broadcast_to((P, F, G, V))

            def bcr(ap, F):      # [128,G,V] -> [128,F,G,V]
                return ap.unsqueeze(1).broadcast_to((P, F, G, V))

            for lvl in range(NUM_LVLS):
                T, F = T_[lvl], F_[lvl]
                po = PX_OFF[lvl]
                cxA = XS[:, SX_OFF[lvl]:SX_OFF[lvl] + T]
                cyA = CYA[:, cy_pos[lvl]:cy_pos[lvl] + T]
                spwA = XPF[:, po + 0 * T: po + 1 * T]
                sphA = XPF[:, po + 1 * T: po + 2 * T]
                lpA = XPF[:, po + 2 * T: po + 3 * T]
                ctA = XPF[:, po + 3 * T: po + 4 * T]
                hw9 = XS[:, SHW_OFF[lvl]:SHW_OFF[lvl] + V]
                hh9 = XS[:, SHH_OFF[lvl]:SHH_OFF[lvl] + V]
                ras = XCB[:, CRAS_OFF[lvl]:CRAS_OFF[lvl] + G * V].rearrange(
                    "p (g v) -> p g v", v=V)

                MLW = kp.tile([P, T], f32, name=f"mlw{lvl}", tag=f"mlw{lvl}")
                MLH = kp.tile([P, T], f32, name=f"mlh{lvl}", tag=f"mlh{lvl}")
                POS = kp.tile([P, T], f32, name=f"pos{lvl}", tag=f"pos{lvl}")

                for f0 in range(0, T, F):
                    cx = cxA[:, f0:f0 + F]
                    cy = cyA[:, f0:f0 + F]

                    dx1 = smp.tile([P, F, G], f32, name="dx1", tag="dx1")
                    dx2 = smp.tile([P, F, G], f32, name="dx2", tag="dx2")
                    dy1 = smp.tile([P, F, G], f32, name="dy1", tag="dy1")
                    dy2 = smp.tile([P, F, G], f32, name="dy2", tag="dy2")
                    nc.gpsimd.tensor_tensor(out=dx1[:, :F], in0=bcc(cx, F), in1=bcg(gx1, F), op=AL.subtract)
                    nc.gpsimd.tensor_tensor(out=dx2[:, :F], in0=bcg(gx2, F), in1=bcc(cx, F), op=AL.subtract)
                    nc.gpsimd.tensor_tensor(out=dy1[:, :F], in0=bcc(cy, F), in1=bcg(gy1, F), op=AL.subtract)
                    nc.gpsimd.tensor_tensor(out=dy2[:, :F], in0=bcg(gy2, F), in1=bcc(cy, F), op=AL.subtract)

                    t1 = bigp.tile([P, F, G, V], f32, name="t1", tag="t1")
                    t2 = bigp.tile([P, F, G, V], f32, name="t2", tag="t2")
                    ix = bigp.tile([P, F, G, V], f32, name="ix", tag="ix")
                    t3 = bigp.tile([P, F, G, V], f32, name="t3", tag="t3")
                    t4 = bigp.tile([P, F, G, V], f32, name="t4", tag="t4")
                    iy = bigp.tile([P, F, G, V], f32, name="iy", tag="iy")
                    iy2 = bigp.tile([P, F, G, V], f32, name="iy2", tag="iy2")
                    rr = bigp.tile([P, F, G, V], f32, name="rr", tag="rr")

                    nc.vector.tensor_tensor(out=t3[:, :F], in0=bcv(hh9, F), in1=bcd(dy1[:, :F], F), op=AL.min)
                    nc.vector.tensor_tensor(out=t4[:, :F], in0=bcv(hh9, F), in1=bcd(dy2[:, :F], F), op=AL.min)
                    nc.gpsimd.tensor_tensor(out=iy[:, :F], in0=t3[:, :F], in1=t4[:, :F], op=AL.add)
                    nc.vector.tensor_tensor(out=t1[:, :F], in0=bcv(hw9, F), in1=bcd(dx1[:, :F], F), op=AL.min)
                    nc.vector.tensor_tensor(out=t2[:, :F], in0=bcv(hw9, F), in1=bcd(dx2[:, :F], F), op=AL.min)
                    nc.gpsimd.tensor_tensor(out=ix[:, :F], in0=t1[:, :F], in1=t2[:, :F], op=AL.add)
                    nc.gpsimd.tensor_tensor(out=iy2[:, :F], in0=iy[:, :F], in1=bcr(ras, F), op=AL.mult)
                    # rr = max(ix, 0) * (iy * ras); negative iy never crosses
                    # the threshold nor beats any positive candidate.
                    nc.vector.scalar_tensor_tensor(
                        out=rr[:, :F], in0=ix[:, :F], scalar=0.0, in1=iy2[:, :F],
                        op0=AL.max, op1=AL.mult)

                    miou = smp.tile([P, F, G], f32, name="miou", tag="miou")
                    nc.vector.reduce_max(out=miou[:, :F], in_=rr[:, :F], axis=AX.X)
                    maxg = smp.tile([P, F], f32, name="maxg", tag="maxg")
                    nc.vector.reduce_max(out=maxg[:, :F], in_=miou[:, :F], axis=AX.X)
                    nc.gpsimd.tensor_single_scalar(
                        out=POS[:, f0:f0 + F], in_=maxg[:, :F], scalar=THRESH, op=AL.is_ge)

                    eq = smp.tile([P, F, G], f32, name="eq", tag="eq")
                    nc.vector.tensor_tensor(
                        out=eq[:, :F], in0=miou[:, :F],
                        in1=maxg[:, :F].unsqueeze(2).broadcast_to((P, F, G)), op=AL.is_equal)
                    cnt = smp.tile([P, F], f32, name="cnt", tag="cnt")
                    nc.vector.reduce_sum(out=cnt[:, :F], in_=eq[:, :F], axis=AX.X)
                    wn = smp.tile([P, F, G], f32, name="wn", tag="wn")
                    hn = smp.tile([P, F, G], f32, name="hn", tag="hn")
                    nc.gpsimd.tensor_tensor(out=wn[:, :F], in0=eq[:, :F], in1=bcg(lgw, F), op=AL.mult)
                    nc.gpsimd.tensor_tensor(out=hn[:, :F], in0=eq[:, :F], in1=bcg(lgh, F), op=AL.mult)
                    wnum = smp.tile([P, F], f32, name="wnum", tag="wnum")
                    hnum = smp.tile([P, F], f32, name="hnum", tag="hnum")
                    nc.vector.reduce_sum(out=wnum[:, :F], in_=wn[:, :F], axis=AX.X)
                    nc.vector.reduce_sum(out=hnum[:, :F], in_=hn[:, :F], axis=AX.X)
                    rc = smp.tile([P, F], f32, name="rc", tag="rc")
                    nc.vector.reciprocal(out=rc[:, :F], in_=cnt[:, :F])
                    nc.gpsimd.tensor_tensor(out=MLW[:, f0:f0 + F], in0=wnum[:, :F], in1=rc[:, :F], op=AL.mult)
                    nc.gpsimd.tensor_tensor(out=MLH[:, f0:f0 + F], in0=hnum[:, :F], in1=rc[:, :F], op=AL.mult)

                # ---------------- phase B: focal + shape loss tails ----------
                sg = pbp.tile([P, T], f32, name="sg", tag="sg")
                nc.scalar.activation(out=sg[:], in_=lpA, func=AF.Sigmoid)
                a1 = pbp.tile([P, T], f32, name="a1", tag="a1")
                nc.scalar.activation(out=a1[:], in_=sg[:], func=AF.Copy, bias=1.0, scale=-2.0)
                ptm = pbp.tile([P, T], f32, name="ptm", tag="ptm")
                nc.gpsimd.tensor_tensor(out=ptm[:], in0=ctA, in1=a1[:], op=AL.mult)
                pt = pbp.tile([P, T], f32, name="pt", tag="pt")
                nc.gpsimd.tensor_tensor(out=pt[:], in0=ptm[:], in1=sg[:], op=AL.add)
                ptc = pbp.tile([P, T], f32, name="ptc", tag="ptc")
                nc.gpsimd.tensor_single_scalar(out=ptc[:], in_=pt[:], scalar=1e-6, op=AL.max)
                lg = pbp.tile([P, T], f32, name="lg", tag="lg")
                nc.scalar.activation(out=lg[:], in_=ptc[:], func=AF.Ln)
                om2 = pbp.tile([P, T], f32, name="om2", tag="om2")
                nc.scalar.activation(out=om2[:], in_=pt[:], func=AF.Square, bias=1.0, scale=-1.0)
                s1 = pbp.tile([P, T], f32, name="s1", tag="s1")
                nc.gpsimd.tensor_tensor(out=s1[:], in0=om2[:], in1=lg[:], op=AL.mult)
                at = pbp.tile([P, T], f32, name="at", tag="at")
                nc.gpsimd.tensor_scalar(at[:], ctA, 0.5, 0.25, AL.mult, AL.add)
                s2 = pbp.tile([P, T], f32, name="s2", tag="s2")
                nc.gpsimd.tensor_tensor(out=s2[:], in0=at[:], in1=s1[:], op=AL.mult)
                nc.vector.reduce_sum(
                    out=ACC[:, 3 * lvl:3 * lvl + 1], in_=s2[:], axis=AX.X)

                slo = []
                for ax, (spA, ML) in enumerate(((spwA, MLW), (sphA, MLH))):
                    lpw = pbp.tile([P, T], f32, name=f"lpw{ax}", tag=f"lpw{ax}")
                    nc.gpsimd.tensor_scalar(lpw[:], spA, 4.0, LOG_S[lvl], AL.min, AL.add)
                    dwm = pbp.tile([P, T], f32, name=f"dwm{ax}", tag=f"dwm{ax}")
                    nc.vector.scalar_tensor_tensor(
                        out=dwm[:], in0=lpw[:], scalar=0.0, in1=ML[:],
                        op0=AL.max, op1=AL.subtract)
                    dw = pbp.tile([P, T], f32, name=f"dw{ax}", tag=f"dw{ax}")
                    nc.scalar.activation(out=dw[:], in_=dwm[:], func=AF.Abs)
                    ee = pbp.tile([P, T], f32, name=f"ee{ax}", tag=f"ee{ax}")
                    nc.scalar.activation(out=ee[:], in_=dw[:], func=AF.Exp, scale=-1.0)
                    c1 = pbp.tile([P, T], f32, name=f"c1{ax}", tag=f"c1{ax}")
                    nc.gpsimd.tensor_single_scalar(out=c1[:], in_=ee[:], scalar=0.8, op=AL.max)
                    u2s = pbp.tile([P, T], f32, name=f"u2s{ax}", tag=f"u2s{ax}")
                    nc.scalar.activation(out=u2s[:], in_=c1[:], func=AF.Square, bias=1.0, scale=-1.0)
                    d1 = pbp.tile([P, T], f32, name=f"d1{ax}", tag=f"d1{ax}")
                    nc.gpsimd.tensor_tensor(out=d1[:], in0=c1[:], in1=ee[:], op=AL.subtract)
                    sl = pbp.tile([P, T], f32, name=f"sl{ax}", tag=f"sl{ax}")
                    nc.vector.scalar_tensor_tensor(
                        out=sl[:], in0=u2s[:], scalar=2.5, in1=d1[:],
                        op0=AL.mult, op1=AL.add)
                    slo.append(sl)
                ssum = pbp.tile([P, T], f32, name="ssum", tag="ssum")
                nc.gpsimd.tensor_tensor(out=ssum[:], in0=slo[0][:], in1=slo[1][:], op=AL.add)
                spm = pbp.tile([P, T], f32, name="spm", tag="spm")
                nc.gpsimd.tensor_tensor(out=spm[:], in0=ssum[:], in1=POS[:], op=AL.mult)
                nc.vector.reduce_sum(
                    out=ACC[:, 3 * lvl + 1:3 * lvl + 2], in_=spm[:], axis=AX.X)
                nc.vector.reduce_sum(out=ACC[:, 3 * lvl + 2:3 * lvl + 3], in_=POS[:], axis=AX.X)

            nc.sync.dma_start(out=OUT[:], in_=ACC[:])
    nc.compile()
    _CACHE["nc"] = nc
    return nc


# ---------------------------------------------------------------- dispatcher
def _dispatcher():
    """Build (once) the cached jitted shard_map dispatcher + resident xs."""
    if "disp" in _CACHE:
        return _CACHE["disp"]
    import jax
    from jax.sharding import Mesh, PartitionSpec, NamedSharding
    from jax.experimental.shard_map import shard_map
    from concourse import mybir
    from concourse.bass2jax import _bass_exec_p, install_neuronx_cc_hook

    nc = _build()
    install_neuronx_cc_hook()

    partition_name = nc.partition_id_tensor.name if nc.partition_id_tensor else None
    in_names, out_names, out_avals = [], [], []
    for alloc in nc.m.functions[0].allocations:
        if not isinstance(alloc, mybir.MemoryLocationSet):
            continue
        name = alloc.memorylocations[0].name
        if alloc.kind == "ExternalInput":
            if name != partition_name:
                in_names.append(name)
        elif alloc.kind == "ExternalOutput":
            out_avals.append(jax.core.ShapedArray(
                tuple(alloc.tensor_shape), mybir.dt.np(alloc.dtype)))
            out_names.append(name)
    in_names_all = list(in_names)
    if partition_name is not None:
        in_names_all.append(partition_name)

    def _body(*args):
        operands = list(args)
        if partition_name is not None:
            from concourse.bass2jax import partition_id_tensor
            operands.append(partition_id_tensor())
        outs = _bass_exec_p.bind(
            *operands,
            out_avals=tuple(out_avals), in_names=tuple(in_names_all),
            out_names=tuple(out_names), lowering_input_output_aliases=(),
            sim_require_finite=True, sim_require_nnan=True, nc=nc)
        return tuple(outs)

    devices = jax.devices()[:N_CORES]
    mesh = Mesh(np.asarray(devices), ("core",))
    in_specs = (PartitionSpec("core"),) * len(in_names)
    out_specs = (PartitionSpec("core"),) * len(out_names)
    sharded = jax.jit(shard_map(
        _body, mesh=mesh, in_specs=in_specs, out_specs=out_specs,
        check_rep=False))

    xs_np = np.broadcast_to(_static_block()[None], (N_CORES, P, NSC))
    xs_np = np.ascontiguousarray(xs_np).reshape(N_CORES * P, NSC)
    xs_dev = jax.device_put(xs_np, NamedSharding(mesh, PartitionSpec("core")))
    jax.block_until_ready(xs_dev)

    order = {n: i for i, n in enumerate(in_names)}
    _CACHE["disp"] = (sharded, xs_dev, order)
    return _CACHE["disp"]


# ---------------------------------------------------------------- emulation
def _emulate_core(xs_blk, xp_core, xc_row):
    """numpy mirror of the device program -> [128, 12]."""
    XS = xs_blk.astype(np.float32)
    XPF = xp_core.astype(np.float32)
    XCB = np.broadcast_to(xc_row[None, :], (P, NCC)).astype(np.float32)
    acc = np.zeros((P, 12), np.float32)
    gx1 = XCB[:, GX1_OFF:GX1_OFF + G]
    gy1 = XCB[:, GY1_OFF:GY1_OFF + G]
    gx2 = XCB[:, GX2_OFF:GX2_OFF + G]
    gy2 = XCB[:, GY2_OFF:GY2_OFF + G]
    lgw = XCB[:, LGW_OFF:LGW_OFF + G]
    lgh = XCB[:, LGH_OFF:LGH_OFF + G]
    for lvl in range(NUM_LVLS):
        T = T_[lvl]
        po = PX_OFF[lvl]
        cx = XS[:, SX_OFF[lvl]:SX_OFF[lvl] + T]
        cy = (XS[:, SX_OFF[lvl] + T:SX_OFF[lvl] + 2 * T]
              + XCB[:, CYOFF_COL:CYOFF_COL + 1])
        spw = XPF[:, po:po + T]
        sph = XPF[:, po + T:po + 2 * T]
        lp = XPF[:, po + 2 * T:po + 3 * T]
        ct = XPF[:, po + 3 * T:po + 4 * T]
        hw9 = XS[:, SHW_OFF[lvl]:SHW_OFF[lvl] + V]
        hh9 = XS[:, SHH_OFF[lvl]:SHH_OFF[lvl] + V]
        ras = XCB[:, CRAS_OFF[lvl]:CRAS_OFF[lvl] + G * V].reshape(P, G, V)

        dx1 = cx[:, :, None] - gx1[:, None, :]
        dx2 = gx2[:, None, :] - cx[:, :, None]
        dy1 = cy[:, :, None] - gy1[:, None, :]
        dy2 = gy2[:, None, :] - cy[:, :, None]
        t1 = np.minimum(hw9[:, None, None, :], dx1[..., None])
        t2 = np.minimum(hw9[:, None, None, :], dx2[..., None])
        ixv = t1 + t2
        t3 = np.minimum(hh9[:, None, None, :], dy1[..., None])
        t4 = np.minimum(hh9[:, None, None, :], dy2[..., None])
        iyv = t3 + t4
        iy2 = iyv * ras[:, None, :, :]
        rrv = np.maximum(ixv, np.float32(0)) * iy2
        miou = rrv.max(axis=3)
        maxg = miou.max(axis=2)
        pos = (maxg >= np.float32(THRESH)).astype(np.float32)
        eq = (miou == maxg[:, :, None]).astype(np.float32)
        cnt = eq.sum(axis=2, dtype=np.float32)
        wnum = (eq * lgw[:, None, :]).sum(axis=2, dtype=np.float32)
        hnum = (eq * lgh[:, None, :]).sum(axis=2, dtype=np.float32)
        rcv = np.float32(1.0) / cnt
        mlw = wnum * rcv
        mlh = hnum * rcv

        # phase B
        sg = np.float32(1.0) / (np.float32(1.0) + np.exp(-lp, dtype=np.float32))
        a1 = np.float32(1.0) - np.float32(2.0) * sg
        pt = ct * a1 + sg
        ptc = np.maximum(pt, np.float32(1e-6))
        lgv = np.log(ptc, dtype=np.float32)
        om2 = np.square(np.float32(1.0) - pt)
        s1 = om2 * lgv
        at = np.float32(0.25) + np.float32(0.5) * ct
        acc[:, 3 * lvl] = (at * s1).sum(axis=1, dtype=np.float32)

        sls = []
        for spA, ML in ((spw, mlw), (sph, mlh)):
            lpw = np.minimum(spA, np.float32(4.0)) + np.float32(LOG_S[lvl])
            dwm = np.maximum(lpw, np.float32(0.0)) - ML
            dwv = np.abs(dwm)
            ee = np.exp(-dwv, dtype=np.float32)
            c1 = np.maximum(ee, np.float32(0.8))
            u2s = np.square(np.float32(1.0) - c1)
            d1 = c1 - ee
            sls.append(np.float32(2.5) * u2s + d1)
        ssum = sls[0] + sls[1]
        acc[:, 3 * lvl + 1] = (ssum * pos).sum(axis=1, dtype=np.float32)
        acc[:, 3 * lvl + 2] = pos.sum(axis=1, dtype=np.float32)
    return acc


# ---------------------------------------------------------------- entry
def _combine(parts):
    s = parts.astype(np.float64).sum(axis=(0, 1))  # [12]
    loc, shp = 0.0, 0.0
    for lvl in range(NUM_LVLS):
        fh, fw = FEAT[lvl]
        loc += (-s[3 * lvl]) / (B * fh * fw)
        shp += s[3 * lvl + 1] / max(4.0 * s[3 * lvl + 2], 1.0)
    return np.array((loc + shp) / NUM_LVLS, dtype=np.float32)


def kernel(**inputs):
    gt = np.asarray(inputs["gt_boxes"], dtype=np.float32)
    loc_preds = [np.asarray(inputs[f"loc_pred{l}"], dtype=np.float32)
                 for l in range(NUM_LVLS)]
    shape_preds = [np.asarray(inputs[f"shape_pred{l}"], dtype=np.float32)
                   for l in range(NUM_LVLS)]
    xp_bf, xc = _host_prep(gt, loc_preds, shape_preds)

    if os.environ.get("KERNEL_EMULATE"):
        xs_blk = _static_block()
        parts = np.stack([
            _emulate_core(xs_blk, xp_bf[c * P:(c + 1) * P].astype(np.float32),
                          xc[c])
            for c in range(N_CORES)])
        return _combine(parts)

    sharded, xs_dev, order = _dispatcher()
    args = [None] * len(order)
    args[order["xs"]] = xs_dev
    args[order["xp"]] = xp_bf
    args[order["xc"]] = xc
    out_arrs = sharded(*args)
    parts = np.asarray(out_arrs[0]).reshape(N_CORES, P, 12)
    return _combine(parts)
